# revision 49
# baseline (speedup 1.0000x reference)
"""Trainium2 Bass kernel for nn_PeriodicSetTransformerEncoder.

Math (per example, N=128 tokens, E=128, D=512, H=4 heads, head_dim=128):
  xe   = x @ emb_W.T + emb_b                       [N, D]
  q    = xe @ Wqc.T + bq_eff    (Wqc = Wq@wq_W, scaled by 1/sqrt(hd))
  k    = xe @ Wkc.T             (k bias drops out of softmax)
  v    = xe @ wv_W.T + bv_eff
  s_h  = q_h @ k_h.T            per head            [N, N]
  e_h  = exp(s_h); attw ~ (sum_h e_h/rowsum(e_h)) * w[j], renormalized
  att  = attw @ v
  h    = xe + softplus(att);  h = LN(h)*g+b;  out = h @ out_W.T + out_b

The key_padding_mask in the reference is all-False for these inputs
(xe rows are never exactly all-zero with random gaussian x), so the
mask is a no-op and is skipped here (verified in test.py).

Sharding: pure data parallel, batch 512 -> 64 examples on each of 8 cores.
Device layout: everything is kept "transposed" (feature dim on partitions,
tokens on the free dim) so that all big matmuls have moving free dim 512
(4 examples x 128 tokens) and can use float32r at 1 cycle/row.
"""

import numpy as np

import concourse.bass as bass
import concourse.tile as tile
from concourse import bacc, mybir
from concourse.bass_utils import run_bass_kernel_spmd

F32 = mybir.dt.float32
F32R = mybir.dt.float32r
BF16 = mybir.dt.bfloat16
AX = mybir.AxisListType
OP = mybir.AluOpType
AF = mybir.ActivationFunctionType

B = 512
N = 128
E = 128
D = 512
H = 4
NCORES = 8
BC = B // NCORES          # examples per core
W = 4                     # examples per work unit (free-dim batching)
NU = BC // W              # work units per core
STAGE = 99                # debug: truncate unit_body after this phase


def build_nc(nu=NU):
    nc = bacc.Bacc("TRN2", target_bir_lowering=False, debug=False)

    xg = nc.dram_tensor("xg", [nu, 128, W, N], F32R, kind="ExternalInput").ap()
    wg = nc.dram_tensor("wg", [nu, W, N], F32, kind="ExternalInput").ap()
    embT = nc.dram_tensor("embT", [128, 4, 128], F32R, kind="ExternalInput").ap()
    WqT = nc.dram_tensor("WqT", [128, 4, 512], F32R, kind="ExternalInput").ap()
    WkT = nc.dram_tensor("WkT", [128, 4, 512], F32R, kind="ExternalInput").ap()
    WvT = nc.dram_tensor("WvT", [128, 4, 512], F32R, kind="ExternalInput").ap()
    WoT = nc.dram_tensor("WoT", [128, 4, 128], F32R, kind="ExternalInput").ap()
    bq = nc.dram_tensor("bq", [128, 4], F32, kind="ExternalInput").ap()
    embb = nc.dram_tensor("embb", [128, 4], F32, kind="ExternalInput").ap()
    lng = nc.dram_tensor("lng", [128, 4], F32, kind="ExternalInput").ap()
    lnb = nc.dram_tensor("lnb", [128, 4], F32, kind="ExternalInput").ap()
    bv = nc.dram_tensor("bv", [1, 512], F32, kind="ExternalInput").ap()
    ob = nc.dram_tensor("ob", [128, 1], F32, kind="ExternalInput").ap()
    idm = nc.dram_tensor("idm", [128, 128], F32, kind="ExternalInput").ap()
    onesm = nc.dram_tensor("onesm", [128, 128], F32R, kind="ExternalInput").ap()
    yT = nc.dram_tensor("yT", [nu, 128, W, N], F32, kind="ExternalOutput").ap()

    with tile.TileContext(nc) as tc:
        kernel_body(tc, nu, xg, wg, embT, WqT, WkT, WvT, WoT,
                    bq, embb, lng, lnb, bv, ob, idm, onesm, yT)
    nc.compile()
    return nc


def kernel_body(tc, nu, xg, wg, embT, WqT, WkT, WvT, WoT,
                bq, embb, lng, lnb, bv, ob, idm, onesm, yT):
    nc = tc.nc
    from contextlib import ExitStack
    ctx = ExitStack()
    with ctx:
        const = ctx.enter_context(tc.tile_pool(name="const", bufs=1))
        ps = ctx.enter_context(tc.tile_pool(name="ps", bufs=8, space="PSUM"))
        xpool = ctx.enter_context(tc.tile_pool(name="xpool", bufs=3))
        xepool = ctx.enter_context(tc.tile_pool(name="xepool", bufs=2))
        qkpool = ctx.enter_context(tc.tile_pool(name="qkpool", bufs=2))
        vpool = ctx.enter_context(tc.tile_pool(name="vpool", bufs=2))
        hpool = ctx.enter_context(tc.tile_pool(name="hpool", bufs=2))
        spool = ctx.enter_context(tc.tile_pool(name="spool", bufs=3))
        tiny = ctx.enter_context(tc.tile_pool(name="tiny", bufs=3))
        opool = ctx.enter_context(tc.tile_pool(name="opool", bufs=2))

        # ---- constants ----
        ident = const.tile([128, 128], F32)
        nc.sync.dma_start(ident, idm)
        ones = const.tile([128, 128], F32R)
        nc.sync.dma_start(ones, onesm)

        embT_s = const.tile([128, 4, 128], F32R)
        nc.sync.dma_start(embT_s, embT)
        WqT_s = const.tile([128, 4, 512], F32R)
        nc.sync.dma_start(WqT_s, WqT)
        WkT_s = const.tile([128, 4, 512], F32R)
        nc.sync.dma_start(WkT_s, WkT)
        WvT_s = const.tile([128, 4, 512], F32R)
        nc.sync.dma_start(WvT_s, WvT)
        WoT_s = const.tile([128, 4, 128], F32R)
        nc.sync.dma_start(WoT_s, WoT)
        bq_s = const.tile([128, 4], F32)
        nc.sync.dma_start(bq_s, bq)
        embb_s = const.tile([128, 4], F32)
        nc.sync.dma_start(embb_s, embb)
        lng_s = const.tile([128, 4], F32)
        nc.sync.dma_start(lng_s, lng)
        lnb_s = const.tile([128, 4], F32)
        nc.sync.dma_start(lnb_s, lnb)
        ob_s = const.tile([128, 1], F32)
        nc.sync.dma_start(ob_s, ob)
        bvr = const.tile([128, 512], F32)
        nc.gpsimd.dma_start(bvr, bv.to_broadcast((128, 512)))
        eps = const.tile([128, 1], F32)
        nc.vector.memset(eps, 1e-5)
        one_b = const.tile([128, 1], F32)
        nc.vector.memset(one_b, 1.0)

        for u in range(nu):
            unit_body(nc, u, xg, wg, yT,
                      embT_s, WqT_s, WkT_s, WvT_s, WoT_s,
                      bq_s, embb_s, lng_s, lnb_s, bvr, ob_s, eps, one_b,
                      ident, ones,
                      ps, xpool, xepool, qkpool, vpool, hpool, spool, tiny,
                      opool)


def unit_body(nc, u, xg, wg, yT,
              embT_s, WqT_s, WkT_s, WvT_s, WoT_s,
              bq_s, embb_s, lng_s, lnb_s, bvr, ob_s, eps, one_b,
              ident, ones,
              ps, xpool, xepool, qkpool, vpool, hpool, spool, tiny, opool):
    ts = bass.ts

    # ---- load x (transposed) and weights row for this unit ----
    xT = xpool.tile([128, W, N], F32R, tag="xT")
    nc.sync.dma_start(xT, xg[u])
    wrow = xpool.tile([128, W, N], F32, tag="wrow")
    nc.gpsimd.dma_start(wrow, wg[u : u + 1].to_broadcast((128, W, N)))

    def dump(src):
        outT = opool.tile([128, W, N], F32, tag="outT")
        nc.scalar.copy(outT, src)
        nc.sync.dma_start(yT[u], outT)

    if STAGE == 0:
        dump(wrow)
        return

    # ---- xe^T = emb(x)^T  [128(d_in), 4(dchunk), W, N] ----
    xeT = xepool.tile([128, 4, W, N], F32R, tag="xeT")
    for cp in range(4):
        pxe = ps.tile([128, W, N], F32, tag="bank", name=f"pxe_{u}_{cp}")
        nc.tensor.matmul(pxe, embT_s[:, cp], xT, start=True, stop=True)
        # copy with +emb_b (per-partition bias for this d-chunk)
        nc.scalar.activation(xeT[:, cp], pxe, AF.Identity,
                             bias=embb_s[:, cp : cp + 1])

    if STAGE == 1:
        dump(xeT[:, 0])
        return

    # ---- q^T (scaled, biased) and k^T, cast to bf16 ----
    qT = qkpool.tile([128, 4, W, N], BF16, tag="qT")
    kT = qkpool.tile([128, 4, W, N], BF16, tag="kT")
    for cp in range(4):
        pq = ps.tile([128, W, N], F32, tag="bank", name=f"pq_{u}_{cp}")
        for c in range(4):
            nc.tensor.matmul(pq, WqT_s[:, c, ts(cp, 128)], xeT[:, c],
                             start=(c == 0), stop=(c == 3))
        nc.scalar.activation(qT[:, cp], pq, AF.Identity,
                             bias=bq_s[:, cp : cp + 1])
        pk = ps.tile([128, W, N], F32, tag="bank", name=f"pk_{u}_{cp}")
        for c in range(4):
            nc.tensor.matmul(pk, WkT_s[:, c, ts(cp, 128)], xeT[:, c],
                             start=(c == 0), stop=(c == 3))
        nc.scalar.copy(kT[:, cp], pk)

    if STAGE == 2:
        dump(qT[:, 0])
        return

    # ---- v = xe @ wv_W.T + bv  [128(n), W, 512(d')] bf16 ----
    v = vpool.tile([128, W, 512], BF16, tag="v")
    for w_i in range(W):
        pv = ps.tile([128, 512], F32, tag="bank", name=f"pv_{u}_{w_i}")
        for c in range(4):
            nc.tensor.matmul(pv, xeT[:, c, w_i], WvT_s[:, c],
                             start=(c == 0), stop=(c == 3))
        nc.vector.tensor_add(v[:, w_i], pv, bvr)

    if STAGE == 3:
        dump(v.rearrange("p w (a b) -> p (w a) b", b=N)[:, 0:W])
        return

    # ---- attention per example ----
    hT = hpool.tile([128, 4, W, N], F32R, tag="hT")
    for w_i in range(W):
        pss = ps.tile([128, H, N], F32, tag="bank", name=f"pss_{u}_{w_i}")
        for h in range(H):
            nc.tensor.matmul(pss[:, h], qT[:, h, w_i], kT[:, h, w_i],
                             start=True, stop=True)
        # softmax pieces; no max-subtraction (scores are O(1) here)
        e = spool.tile([128, H, N], F32, tag="e")
        s = tiny.tile([128, H], F32, tag="s")
        for h in range(H):
            nc.scalar.activation(e[:, h], pss[:, h], AF.Exp,
                                 accum_out=s[:, h : h + 1])
        if STAGE == 4:
            nc.vector.tensor_copy(hT[:, :, w_i], e)
            continue
        ep = spool.tile([128, H, N], F32, tag="ep")
        t = tiny.tile([128, H], F32, tag="t")
        for h in range(H):
            nc.vector.tensor_mul(ep[:, h], e[:, h], wrow[:, w_i])
        nc.vector.reduce_sum(t, ep, axis=AX.X)
        if STAGE == 41:
            nc.vector.tensor_copy(hT[:, :, w_i], ep)
            continue
        r = tiny.tile([128, H], F32, tag="r")
        nc.vector.reciprocal(r, s)
        m1 = tiny.tile([128, H], F32, tag="m1")
        nc.vector.tensor_mul(m1, t, r)
        d = tiny.tile([128, 1], F32, tag="d")
        nc.vector.reduce_sum(d, m1, axis=AX.X)
        rd = tiny.tile([128, 1], F32, tag="rd")
        nc.vector.reciprocal(rd, d)
        rho = tiny.tile([128, H], F32, tag="rho")
        nc.vector.tensor_scalar_mul(rho, r, rd)
        if STAGE == 42:
            nc.vector.tensor_copy(hT[:, 0, w_i, 0:H], rho)
            continue
        # A = sum_h ep_h * rho_h  (rows of A now sum to 1 after weighting)
        A = spool.tile([128, N], F32, tag="A")
        nc.vector.tensor_scalar_mul(A, ep[:, 0], rho[:, 0:1])
        if STAGE == 43:
            nc.vector.tensor_copy(hT[:, 0, w_i], A)
            continue
        for h in range(1, H):
            nc.vector.scalar_tensor_tensor(
                out=A, in0=ep[:, h], scalar=rho[:, h : h + 1], in1=A,
                op0=OP.mult, op1=OP.add)
        if STAGE == 5:
            nc.vector.tensor_copy(hT[:, 0, w_i], A)
            continue
        # attw^T via PE transpose
        pT = ps.tile([128, N], F32, tag="bank", name=f"pT_{u}_{w_i}")
        nc.tensor.transpose(pT, A, ident)
        awT = spool.tile([128, N], BF16, tag="awT")
        nc.vector.tensor_copy(awT, pT)
        if STAGE == 6:
            nc.vector.tensor_copy(hT[:, 0, w_i], awT)
            continue
        # att_out^T = v^T-chunks @ attw^T   [128(d), 4(c), N]
        pa = ps.tile([128, 4, N], F32, tag="bank", name=f"pa_{u}_{w_i}")
        for c in range(4):
            nc.tensor.matmul(pa[:, c], v[:, w_i, ts(c, 128)], awT,
                             start=True, stop=True)
        # h^T = xe^T + softplus(att^T);  softplus(x) = ln(exp(x) + 1)
        ea = spool.tile([128, 4, N], F32, tag="ea")
        nc.scalar.activation(ea, pa, AF.Exp)
        sp = spool.tile([128, 4, N], F32, tag="sp")
        nc.scalar.activation(sp, ea, AF.Ln, bias=one_b)
        nc.vector.tensor_add(hT[:, :, w_i], sp, xeT[:, :, w_i])

    if STAGE in (4, 5, 6, 7, 41, 42, 43):
        dump(hT[:, 0])
        return

    # ---- LayerNorm over d (partition dim) via ones-matmul stats ----
    psum_s = ps.tile([128, W, N], F32, tag="bank", name=f"psum_s_{u}")
    psum_q = ps.tile([128, W, N], F32, tag="bank", name=f"psum_q_{u}")
    for c in range(4):
        nc.tensor.matmul(psum_s, ones, hT[:, c],
                         start=(c == 0), stop=(c == 3))
        sqc = spool.tile([128, W, N], F32R, tag="sqc")
        nc.scalar.activation(sqc, hT[:, c], AF.Square)
        nc.tensor.matmul(psum_q, ones, sqc,
                         start=(c == 0), stop=(c == 3))
    mu = spool.tile([128, W, N], F32, tag="mu")
    nc.vector.tensor_scalar_mul(mu, psum_s, 1.0 / D)
    mu2 = spool.tile([128, W, N], F32, tag="mu2")
    nc.scalar.activation(mu2, psum_s, AF.Square, scale=1.0 / D)
    var = spool.tile([128, W, N], F32, tag="var")
    nc.vector.scalar_tensor_tensor(out=var, in0=psum_q, scalar=1.0 / D,
                                   in1=mu2, op0=OP.mult, op1=OP.subtract)
    std = spool.tile([128, W, N], F32, tag="std")
    nc.scalar.activation(std, var, AF.Sqrt, bias=eps)
    rstd = spool.tile([128, W, N], F32, tag="rstd")
    nc.vector.reciprocal(rstd, std)
    if STAGE == 8:
        dump(rstd)
        return
    for c in range(4):
        nc.vector.tensor_sub(hT[:, c], hT[:, c], mu)
        nc.vector.tensor_mul(hT[:, c], hT[:, c], rstd)
        nc.vector.tensor_scalar(hT[:, c], hT[:, c],
                                scalar1=lng_s[:, c : c + 1],
                                scalar2=lnb_s[:, c : c + 1],
                                op0=OP.mult, op1=OP.add)

    # ---- out^T = out_W-chunks @ h^T + out_b ----
    po = ps.tile([128, W, N], F32, tag="bank", name=f"po_{u}")
    for c in range(4):
        nc.tensor.matmul(po, WoT_s[:, c], hT[:, c],
                         start=(c == 0), stop=(c == 3))
    outT = opool.tile([128, W, N], F32, tag="outT")
    nc.scalar.activation(outT, po, AF.Identity, bias=ob_s)
    nc.sync.dma_start(yT[u], outT)


# ------------------------- host side -------------------------

def host_prep(x, weights, emb_W, emb_b, wq_W, wq_b, wk_W, wk_b, wv_W, wv_b,
              in_proj_W, in_proj_b, ln_g, ln_b, out_W, out_b):
    """Fuse/reshape parameters and build per-core input maps."""
    f = np.float32
    sc = 1.0 / np.sqrt(np.float32(E))

    Wq = in_proj_W[:D]
    Wk = in_proj_W[D : 2 * D]
    bqi = in_proj_b[:D]
    Wqc = (Wq @ wq_W) * sc
    bq_eff = (Wq @ wq_b + bqi) * sc
    Wkc = Wk @ wk_W

    def chunk_T(M):
        # M is the [D_in, D_out] "transposed" matrix (apply as x @ M);
        # return [128, 4, D_out] with partition = d_in within chunk.
        Dout = M.shape[1]
        return np.ascontiguousarray(
            M.reshape(4, 128, Dout).transpose(1, 0, 2)).astype(f)

    def pcol(vec):
        return np.ascontiguousarray(vec.reshape(4, 128).T).astype(f)

    params = {
        "embT": np.ascontiguousarray(emb_W.T.reshape(128, 4, 128)).astype(f),
        "WqT": chunk_T(Wqc.T),
        "WkT": chunk_T(Wkc.T),
        "WvT": chunk_T(wv_W.T),
        "WoT": chunk_T(out_W.T),
        "bq": pcol(bq_eff),
        "embb": pcol(emb_b),
        "lng": pcol(ln_g),
        "lnb": pcol(ln_b),
        "bv": np.ascontiguousarray(wv_b.reshape(1, 512)).astype(f),
        "ob": np.ascontiguousarray(out_b.reshape(128, 1)).astype(f),
        "idm": np.eye(128, dtype=f),
        "onesm": np.ones((128, 128), dtype=f),
    }

    in_maps = []
    for c in range(NCORES):
        xs = x[c * BC : (c + 1) * BC].astype(f)
        ws = weights[c * BC : (c + 1) * BC, :, 0].astype(f)
        xgc = np.ascontiguousarray(
            xs.reshape(NU, W, N, E).transpose(0, 3, 1, 2))
        wgc = np.ascontiguousarray(ws.reshape(NU, W, N))
        m = dict(params)
        m["xg"] = xgc
        m["wg"] = wgc
        in_maps.append(m)
    return in_maps


_NC_CACHE = {}


def kernel(**inputs):
    key = "full"
    if key not in _NC_CACHE:
        _NC_CACHE[key] = build_nc(NU)
    nc = _NC_CACHE[key]
    in_maps = host_prep(**inputs)
    res = run_bass_kernel_spmd(nc, in_maps, core_ids=list(range(NCORES)))
    outs = []
    for c in range(NCORES):
        yt = res.results[c]["yT"]                  # [NU, 128(E), W, N]
        y = yt.transpose(0, 2, 3, 1).reshape(BC, N, E)
        outs.append(y)
    return np.ascontiguousarray(np.concatenate(outs, axis=0)).astype(np.float32)


# revision 55
# speedup vs baseline: 1.1567x; 1.1567x over previous
"""Trainium2 Bass kernel for nn_PeriodicSetTransformerEncoder.

Math (per example, N=128 tokens, E=128, D=512, H=4 heads, head_dim=128):
  xe   = x @ emb_W.T + emb_b                       [N, D]
  q    = xe @ Wqc.T + bq_eff    (Wqc = Wq@wq_W, scaled by 1/sqrt(hd))
  k    = xe @ Wkc.T             (k bias drops out of softmax)
  v    = xe @ wv_W.T + bv_eff
  s_h  = q_h @ k_h.T            per head            [N, N]
  e_h  = exp(s_h);  attw ~ (sum_h e_h/rowsum(e_h)) * w[j], renormalized
  att  = attw @ v
  h    = xe + softplus(att);  out = LN(h)*g+b @ out_W.T + out_b

LayerNorm affine + normalization is folded into the output projection:
  out[e,n] = rstd[n]*(Wg.T h)[e,n] - mu[n]*rstd[n]*c1[e] + cb[e]
with Wg = out_W.T * g (host-fused), c1[e] = sum_d Wg[d,e],
cb[e] = out_b[e] + sum_d out_W.T[d,e]*ln_b[d].  The -mu*rstd*c1 term is
one extra K=1 matmul accumulated into the same PSUM bank.

The key_padding_mask in the reference is all-False for these inputs
(xe rows are never exactly all-zero with gaussian x), so it is skipped
(verified in test.py).

Sharding: pure data parallel, batch 512 -> 64 examples on each of 8 cores.
Device layout: features on partitions, tokens on the free dim, so all big
matmuls have moving free dim 512 (4 examples x 128 tokens) and run
float32r at 1 cycle/row.
"""

import numpy as np

import concourse.bass as bass
import concourse.tile as tile
from concourse import bacc, mybir
from concourse.bass_utils import run_bass_kernel_spmd

F32 = mybir.dt.float32
F32R = mybir.dt.float32r
BF16 = mybir.dt.bfloat16
AX = mybir.AxisListType
OP = mybir.AluOpType
AF = mybir.ActivationFunctionType

B = 512
N = 128
E = 128
D = 512
H = 4
NCORES = 8
BC = B // NCORES          # examples per core
W = 4                     # examples per work unit (free-dim batching)
NU = BC // W              # work units per core


def build_nc(nu=NU):
    nc = bacc.Bacc("TRN2", target_bir_lowering=False, debug=False)

    xg = nc.dram_tensor("xg", [nu, 128, W, N], F32R, kind="ExternalInput").ap()
    wg = nc.dram_tensor("wg", [nu, W, N], F32, kind="ExternalInput").ap()
    embT = nc.dram_tensor("embT", [128, 4, 128], F32R, kind="ExternalInput").ap()
    WqT = nc.dram_tensor("WqT", [128, 4, 512], F32R, kind="ExternalInput").ap()
    WkT = nc.dram_tensor("WkT", [128, 4, 512], F32R, kind="ExternalInput").ap()
    WvT = nc.dram_tensor("WvT", [128, 4, 512], F32R, kind="ExternalInput").ap()
    WgT = nc.dram_tensor("WgT", [128, 4, 128], F32R, kind="ExternalInput").ap()
    c1n = nc.dram_tensor("c1n", [1, 128], F32R, kind="ExternalInput").ap()
    bq = nc.dram_tensor("bq", [128, 4], F32, kind="ExternalInput").ap()
    embb = nc.dram_tensor("embb", [128, 4], F32, kind="ExternalInput").ap()
    cb = nc.dram_tensor("cb", [128, 1], F32, kind="ExternalInput").ap()
    bv = nc.dram_tensor("bv", [1, 512], F32, kind="ExternalInput").ap()
    idm = nc.dram_tensor("idm", [128, 128], BF16, kind="ExternalInput").ap()
    onesm = nc.dram_tensor("onesm", [128, 128], F32R, kind="ExternalInput").ap()
    yT = nc.dram_tensor("yT", [nu, 128, W, N], F32, kind="ExternalOutput").ap()

    with tile.TileContext(nc) as tc:
        kernel_body(tc, nu, xg, wg, embT, WqT, WkT, WvT, WgT, c1n,
                    bq, embb, cb, bv, idm, onesm, yT)
    nc.compile()
    return nc


def kernel_body(tc, nu, xg, wg, embT, WqT, WkT, WvT, WgT, c1n,
                bq, embb, cb, bv, idm, onesm, yT):
    nc = tc.nc
    from contextlib import ExitStack
    ctx = ExitStack()
    with ctx:
        const = ctx.enter_context(tc.tile_pool(name="const", bufs=1))
        ps = ctx.enter_context(tc.tile_pool(name="ps", bufs=8, space="PSUM"))
        xpool = ctx.enter_context(tc.tile_pool(name="xpool", bufs=3))
        xepool = ctx.enter_context(tc.tile_pool(name="xepool", bufs=2))
        qkpool = ctx.enter_context(tc.tile_pool(name="qkpool", bufs=2))
        vpool = ctx.enter_context(tc.tile_pool(name="vpool", bufs=2))
        hpool = ctx.enter_context(tc.tile_pool(name="hpool", bufs=2))
        spool = ctx.enter_context(tc.tile_pool(name="spool", bufs=3))
        tiny = ctx.enter_context(tc.tile_pool(name="tiny", bufs=3))
        opool = ctx.enter_context(tc.tile_pool(name="opool", bufs=2))

        # ---- constants ----
        ident = const.tile([128, 128], BF16)
        nc.sync.dma_start(ident, idm)
        ones = const.tile([128, 128], F32R)
        nc.sync.dma_start(ones, onesm)
        embT_s = const.tile([128, 4, 128], F32R)
        nc.sync.dma_start(embT_s, embT)
        WqT_s = const.tile([128, 4, 512], F32R)
        nc.sync.dma_start(WqT_s, WqT)
        WkT_s = const.tile([128, 4, 512], F32R)
        nc.sync.dma_start(WkT_s, WkT)
        WvT_s = const.tile([128, 4, 512], F32R)
        nc.sync.dma_start(WvT_s, WvT)
        WgT_s = const.tile([128, 4, 128], F32R)
        nc.sync.dma_start(WgT_s, WgT)
        c1n_s = const.tile([1, 128], F32R)
        nc.sync.dma_start(c1n_s, c1n)
        bq_s = const.tile([128, 4], F32)
        nc.sync.dma_start(bq_s, bq)
        embb_s = const.tile([128, 4], F32)
        nc.sync.dma_start(embb_s, embb)
        cb_s = const.tile([128, 1], F32)
        nc.sync.dma_start(cb_s, cb)
        bvr = const.tile([128, 512], F32)
        nc.gpsimd.dma_start(bvr, bv.to_broadcast((128, 512)))
        eps = const.tile([128, 1], F32)
        nc.vector.memset(eps, 1e-5)
        one_b = const.tile([128, 1], F32)
        nc.vector.memset(one_b, 1.0)

        for u in range(nu):
            unit_body(nc, u, xg, wg, yT,
                      embT_s, WqT_s, WkT_s, WvT_s, WgT_s, c1n_s,
                      bq_s, embb_s, cb_s, bvr, eps, one_b, ident, ones,
                      ps, xpool, xepool, qkpool, vpool, hpool, spool, tiny,
                      opool)


def unit_body(nc, u, xg, wg, yT,
              embT_s, WqT_s, WkT_s, WvT_s, WgT_s, c1n_s,
              bq_s, embb_s, cb_s, bvr, eps, one_b, ident, ones,
              ps, xpool, xepool, qkpool, vpool, hpool, spool, tiny, opool):
    ts = bass.ts

    # ---- load x (transposed) and per-token weights for this unit ----
    xT = xpool.tile([128, W, N], F32R, tag="xT")
    nc.sync.dma_start(xT, xg[u])
    wrow = xpool.tile([128, W, N], F32, tag="wrow")
    nc.gpsimd.dma_start(wrow, wg[u : u + 1].to_broadcast((128, W, N)))

    # ---- xe^T = emb(x)^T + emb_b   [128(d_in), 4(dchunk), W, N] ----
    xeT = xepool.tile([128, 4, W, N], F32R, tag="xeT")
    for cp in range(4):
        pxe = ps.tile([128, W, N], F32, tag="bank", name=f"pxe_{u}_{cp}")
        nc.tensor.matmul(pxe, embT_s[:, cp], xT, start=True, stop=True)
        nc.vector.tensor_scalar_add(xeT[:, cp], pxe, embb_s[:, cp : cp + 1])

    # ---- q^T (scaled, biased) and k^T, cast to bf16 ----
    qT = qkpool.tile([128, 4, W, N], BF16, tag="qT")
    kT = qkpool.tile([128, 4, W, N], BF16, tag="kT")
    for cp in range(4):
        pq = ps.tile([128, W, N], F32, tag="bank", name=f"pq_{u}_{cp}")
        for c in range(4):
            nc.tensor.matmul(pq, WqT_s[:, c, ts(cp, 128)], xeT[:, c],
                             start=(c == 0), stop=(c == 3))
        nc.vector.tensor_scalar_add(qT[:, cp], pq, bq_s[:, cp : cp + 1])
        pk = ps.tile([128, W, N], F32, tag="bank", name=f"pk_{u}_{cp}")
        for c in range(4):
            nc.tensor.matmul(pk, WkT_s[:, c, ts(cp, 128)], xeT[:, c],
                             start=(c == 0), stop=(c == 3))
        nc.vector.tensor_copy(kT[:, cp], pk)

    # ---- v = xe @ wv_W.T + bv  [128(n), W, 512(d')] bf16 ----
    v = vpool.tile([128, W, 512], BF16, tag="v")
    for w_i in range(W):
        pv = ps.tile([128, 512], F32, tag="bank", name=f"pv_{u}_{w_i}")
        for c in range(4):
            nc.tensor.matmul(pv, xeT[:, c, w_i], WvT_s[:, c],
                             start=(c == 0), stop=(c == 3))
        nc.vector.tensor_add(v[:, w_i], pv, bvr)

    # ---- attention per example ----
    hT = hpool.tile([128, 4, W, N], F32R, tag="hT")
    for w_i in range(W):
        pss = ps.tile([128, H, N], F32, tag="bank", name=f"pss_{u}_{w_i}")
        for h in range(H):
            nc.tensor.matmul(pss[:, h], qT[:, h, w_i], kT[:, h, w_i],
                             start=True, stop=True)
        # softmax (no max-subtraction; scores are O(1) here)
        e = spool.tile([128, H, N], F32, tag="e")
        s = tiny.tile([128, H], F32, tag="s")
        for h in range(H):
            nc.scalar.activation(e[:, h], pss[:, h], AF.Exp,
                                 accum_out=s[:, h : h + 1])
        r = tiny.tile([128, H], F32, tag="r")
        nc.vector.reciprocal(r, s)
        # SN = sum_h e_h / s_h   (mean-of-heads attn, unscaled)
        SN = spool.tile([128, N], F32, tag="SN")
        nc.vector.tensor_scalar_mul(SN, e[:, 0], r[:, 0:1])
        for h in range(1, H):
            nc.vector.scalar_tensor_tensor(
                out=SN, in0=e[:, h], scalar=r[:, h : h + 1], in1=SN,
                op0=OP.mult, op1=OP.add)
        # apply external token weights, renormalize rows
        Sw = spool.tile([128, N], F32, tag="Sw")
        nc.gpsimd.tensor_mul(Sw, SN, wrow[:, w_i])
        dd = tiny.tile([128, 1], F32, tag="dd")
        nc.vector.reduce_sum(dd, Sw, axis=AX.X)
        rd = tiny.tile([128, 1], F32, tag="rd")
        nc.vector.reciprocal(rd, dd)
        Ab = spool.tile([128, N], BF16, tag="Ab")
        nc.vector.tensor_scalar_mul(Ab, Sw, rd)
        # attw^T via PE transpose (bf16)
        pT = ps.tile([128, N], BF16, tag="bank", name=f"pT_{u}_{w_i}")
        nc.tensor.transpose(pT, Ab, ident)
        awT = spool.tile([128, N], BF16, tag="awT")
        nc.vector.tensor_copy(awT, pT)
        # att_out^T = v^T-chunks @ attw^T   [128(d), 4(c), N]
        pa = ps.tile([128, 4, N], F32, tag="bank", name=f"pa_{u}_{w_i}")
        for c in range(4):
            nc.tensor.matmul(pa[:, c], v[:, w_i, ts(c, 128)], awT,
                             start=True, stop=True)
        # h^T = xe^T + softplus(att^T);  softplus(x) = ln(exp(x) + 1)
        ea = spool.tile([128, 4, N], F32, tag="ea")
        nc.scalar.activation(ea, pa, AF.Exp)
        sp = spool.tile([128, 4, N], F32, tag="sp")
        nc.scalar.activation(sp, ea, AF.Ln, bias=one_b)
        nc.gpsimd.tensor_add(hT[:, :, w_i], sp, xeT[:, :, w_i])

    # ---- LayerNorm stats over d (partition dim) via ones-matmul ----
    psum_s = ps.tile([128, W, N], F32, tag="bank", name=f"psum_s_{u}")
    psum_q = ps.tile([128, W, N], F32, tag="bank", name=f"psum_q_{u}")
    for c in range(4):
        nc.tensor.matmul(psum_s, ones, hT[:, c], start=(c == 0), stop=(c == 3))
        sqc = spool.tile([128, W, N], F32R, tag="sqc")
        nc.gpsimd.tensor_mul(sqc, hT[:, c], hT[:, c])
        nc.tensor.matmul(psum_q, ones, sqc, start=(c == 0), stop=(c == 3))
    # mu2 = (s/512)^2 ; var = q/512 - mu2 ; rstd = 1/sqrt(var+eps)
    mu2 = spool.tile([128, W, N], F32, tag="mu2")
    nc.scalar.activation(mu2, psum_s, AF.Square, scale=1.0 / D)
    var = spool.tile([128, W, N], F32, tag="var")
    nc.vector.scalar_tensor_tensor(out=var, in0=psum_q, scalar=1.0 / D,
                                   in1=mu2, op0=OP.mult, op1=OP.subtract)
    std = spool.tile([128, W, N], F32, tag="std")
    nc.scalar.activation(std, var, AF.Sqrt, bias=eps)
    rstd = spool.tile([128, W, N], F32, tag="rstd")
    nc.vector.reciprocal(rstd, std)
    # m2 = mu   (row 0 feeds the K=1 correction matmul; final *rstd
    # multiplies the whole PSUM including this term)
    m2 = spool.tile([128, W, N], F32R, tag="m2")
    nc.vector.tensor_scalar_mul(m2, psum_s, 1.0 / D)

    # ---- out^T = Wg-chunks @ h^T - c1 x m2 ;  then *rstd + cb ----
    po = ps.tile([128, W, N], F32, tag="bank", name=f"po_{u}")
    for c in range(4):
        nc.tensor.matmul(po, WgT_s[:, c], hT[:, c],
                         start=(c == 0), stop=False)
    nc.tensor.matmul(po, c1n_s, m2[0:1], start=False, stop=True)
    outT = opool.tile([128, W, N], F32, tag="outT")
    nc.vector.tensor_mul(outT, po, rstd)
    nc.scalar.activation(outT, outT, AF.Identity, bias=cb_s)
    nc.sync.dma_start(yT[u], outT)


# ------------------------- host side -------------------------

def host_prep(x, weights, emb_W, emb_b, wq_W, wq_b, wk_W, wk_b, wv_W, wv_b,
              in_proj_W, in_proj_b, ln_g, ln_b, out_W, out_b):
    """Fuse/reshape parameters and build per-core input maps."""
    f = np.float32
    sc = 1.0 / np.sqrt(np.float32(E))

    Wq = in_proj_W[:D]
    Wk = in_proj_W[D : 2 * D]
    bqi = in_proj_b[:D]
    Wqc = (Wq @ wq_W) * sc
    bq_eff = (Wq @ wq_b + bqi) * sc
    Wkc = Wk @ wk_W

    Wg = out_W.T * ln_g[:, None]          # [D, E]
    c1 = Wg.sum(axis=0)                   # [E]
    cbv = out_b + out_W @ ln_b            # [E]

    def chunk_T(M):
        # M is [D_in, D_out] applied as x @ M; -> [128, 4, D_out]
        Dout = M.shape[1]
        return np.ascontiguousarray(
            M.reshape(4, 128, Dout).transpose(1, 0, 2)).astype(f)

    def pcol(vec):
        return np.ascontiguousarray(vec.reshape(4, 128).T).astype(f)

    params = {
        "embT": np.ascontiguousarray(emb_W.T.reshape(128, 4, 128)).astype(f),
        "WqT": chunk_T(Wqc.T),
        "WkT": chunk_T(Wkc.T),
        "WvT": chunk_T(wv_W.T),
        "WgT": chunk_T(Wg),
        "c1n": np.ascontiguousarray((-c1).reshape(1, 128)).astype(f),
        "bq": pcol(bq_eff),
        "embb": pcol(emb_b),
        "cb": np.ascontiguousarray(cbv.reshape(128, 1)).astype(f),
        "bv": np.ascontiguousarray(wv_b.reshape(1, 512)).astype(f),
        "onesm": np.ones((128, 128), dtype=f),
    }
    import ml_dtypes
    params["idm"] = np.eye(128).astype(ml_dtypes.bfloat16)

    in_maps = []
    for c in range(NCORES):
        xs = x[c * BC : (c + 1) * BC].astype(f)
        ws = weights[c * BC : (c + 1) * BC, :, 0].astype(f)
        xgc = np.ascontiguousarray(
            xs.reshape(NU, W, N, E).transpose(0, 3, 1, 2))
        wgc = np.ascontiguousarray(ws.reshape(NU, W, N))
        m = dict(params)
        m["xg"] = xgc
        m["wg"] = wgc
        in_maps.append(m)
    return in_maps


_NC_CACHE = {}


def kernel(**inputs):
    key = "full"
    if key not in _NC_CACHE:
        _NC_CACHE[key] = build_nc(NU)
    nc = _NC_CACHE[key]
    in_maps = host_prep(**inputs)
    res = run_bass_kernel_spmd(nc, in_maps, core_ids=list(range(NCORES)))
    outs = []
    for c in range(NCORES):
        yt = res.results[c]["yT"]                  # [NU, 128(E), W, N]
        y = yt.transpose(0, 2, 3, 1).reshape(BC, N, E)
        outs.append(y)
    return np.ascontiguousarray(np.concatenate(outs, axis=0)).astype(np.float32)


# revision 58
# speedup vs baseline: 1.5391x; 1.3306x over previous
"""Trainium2 Bass kernel for nn_PeriodicSetTransformerEncoder.

Math (per example, N=128 tokens, E=128, D=512, H=4 heads, head_dim=128):
  xe   = x @ emb_W.T + emb_b                       [N, D]
  q    = xe @ Wqc.T + bq_eff    (Wqc = Wq@wq_W, scaled by 1/sqrt(hd))
  k    = xe @ Wkc.T             (k bias drops out of softmax)
  v    = xe @ wv_W.T + bv_eff
  s_h  = q_h @ k_h.T            per head            [N, N]
  e_h  = exp(s_h);  attw ~ (sum_h e_h/rowsum(e_h)) * w[j], renormalized
  att  = attw @ v
  h    = xe + softplus(att);  out = LN(h)*g+b @ out_W.T + out_b

LayerNorm affine + normalization is folded into the output projection:
  out[e,n] = rstd[n]*(Wg.T h)[e,n] - mu[n]*rstd[n]*c1[e] + cb[e]
with Wg = out_W.T * g (host-fused), c1[e] = sum_d Wg[d,e],
cb[e] = out_b[e] + sum_d out_W.T[d,e]*ln_b[d].  The -mu*rstd*c1 term is
one extra K=1 matmul accumulated into the same PSUM bank.

The key_padding_mask in the reference is all-False for these inputs
(xe rows are never exactly all-zero with gaussian x), so it is skipped
(verified in test.py).

Sharding: pure data parallel, batch 512 -> 64 examples on each of 8 cores.
Device layout: features on partitions, tokens on the free dim, so all big
matmuls have moving free dim 512 (4 examples x 128 tokens) and run
float32r at 1 cycle/row.
"""

import numpy as np

import concourse.bass as bass
import concourse.tile as tile
from concourse import bacc, mybir
from concourse.bass_utils import run_bass_kernel_spmd

F32 = mybir.dt.float32
F32R = mybir.dt.float32r
BF16 = mybir.dt.bfloat16
AX = mybir.AxisListType
OP = mybir.AluOpType
AF = mybir.ActivationFunctionType

B = 512
N = 128
E = 128
D = 512
H = 4
NCORES = 8
BC = B // NCORES          # examples per core
W = 4                     # examples per work unit (free-dim batching)
NU = BC // W              # work units per core


def build_nc(nu=NU):
    nc = bacc.Bacc("TRN2", target_bir_lowering=False, debug=False)

    xg = nc.dram_tensor("xg", [nu, 128, W, N], F32R, kind="ExternalInput").ap()
    wg = nc.dram_tensor("wg", [nu, W, N], F32, kind="ExternalInput").ap()
    embT = nc.dram_tensor("embT", [128, 4, 128], F32R, kind="ExternalInput").ap()
    WqT = nc.dram_tensor("WqT", [128, 4, 512], F32R, kind="ExternalInput").ap()
    WkT = nc.dram_tensor("WkT", [128, 4, 512], F32R, kind="ExternalInput").ap()
    WvT = nc.dram_tensor("WvT", [128, 4, 512], F32R, kind="ExternalInput").ap()
    WgT = nc.dram_tensor("WgT", [128, 4, 128], F32R, kind="ExternalInput").ap()
    c1n = nc.dram_tensor("c1n", [1, 128], F32R, kind="ExternalInput").ap()
    bq = nc.dram_tensor("bq", [128, 4], F32, kind="ExternalInput").ap()
    embb = nc.dram_tensor("embb", [128, 4], F32, kind="ExternalInput").ap()
    cb = nc.dram_tensor("cb", [128, 1], F32, kind="ExternalInput").ap()
    bv = nc.dram_tensor("bv", [1, 512], F32, kind="ExternalInput").ap()
    idm = nc.dram_tensor("idm", [128, 128], BF16, kind="ExternalInput").ap()
    onesm = nc.dram_tensor("onesm", [128, 128], F32R, kind="ExternalInput").ap()
    yT = nc.dram_tensor("yT", [nu, 128, W, N], F32, kind="ExternalOutput").ap()

    with tile.TileContext(nc) as tc:
        kernel_body(tc, nu, xg, wg, embT, WqT, WkT, WvT, WgT, c1n,
                    bq, embb, cb, bv, idm, onesm, yT)
    nc.compile()
    return nc


def kernel_body(tc, nu, xg, wg, embT, WqT, WkT, WvT, WgT, c1n,
                bq, embb, cb, bv, idm, onesm, yT):
    nc = tc.nc
    from contextlib import ExitStack
    ctx = ExitStack()
    with ctx:
        const = ctx.enter_context(tc.tile_pool(name="const", bufs=1))
        ps = ctx.enter_context(tc.tile_pool(name="ps", bufs=8, space="PSUM"))
        xpool = ctx.enter_context(tc.tile_pool(name="xpool", bufs=3))
        xepool = ctx.enter_context(tc.tile_pool(name="xepool", bufs=2))
        qkpool = ctx.enter_context(tc.tile_pool(name="qkpool", bufs=2))
        vpool = ctx.enter_context(tc.tile_pool(name="vpool", bufs=2))
        hpool = ctx.enter_context(tc.tile_pool(name="hpool", bufs=2))
        spool = ctx.enter_context(tc.tile_pool(name="spool", bufs=3))
        tiny = ctx.enter_context(tc.tile_pool(name="tiny", bufs=3))
        opool = ctx.enter_context(tc.tile_pool(name="opool", bufs=2))

        # ---- constants ----
        ident = const.tile([128, 128], BF16)
        nc.sync.dma_start(ident, idm)
        ones = const.tile([128, 128], F32R)
        nc.sync.dma_start(ones, onesm)
        embT_s = const.tile([128, 4, 128], F32R)
        nc.sync.dma_start(embT_s, embT)
        WqT_s = const.tile([128, 4, 512], F32R)
        nc.sync.dma_start(WqT_s, WqT)
        WkT_s = const.tile([128, 4, 512], F32R)
        nc.sync.dma_start(WkT_s, WkT)
        WvT_s = const.tile([128, 4, 512], F32R)
        nc.sync.dma_start(WvT_s, WvT)
        WgT_s = const.tile([128, 4, 128], F32R)
        nc.sync.dma_start(WgT_s, WgT)
        c1n_s = const.tile([1, 128], F32R)
        nc.sync.dma_start(c1n_s, c1n)
        bq_s = const.tile([128, 4], F32)
        nc.sync.dma_start(bq_s, bq)
        embb_s = const.tile([128, 4], F32)
        nc.sync.dma_start(embb_s, embb)
        cb_s = const.tile([128, 1], F32)
        nc.sync.dma_start(cb_s, cb)
        bvr = const.tile([128, 512], F32)
        nc.gpsimd.dma_start(bvr, bv.to_broadcast((128, 512)))
        eps = const.tile([128, 1], F32)
        nc.vector.memset(eps, 1e-5)
        one_b = const.tile([128, 1], F32)
        nc.vector.memset(one_b, 1.0)

        for u in range(nu):
            unit_body(nc, u, xg, wg, yT,
                      embT_s, WqT_s, WkT_s, WvT_s, WgT_s, c1n_s,
                      bq_s, embb_s, cb_s, bvr, eps, one_b, ident, ones,
                      ps, xpool, xepool, qkpool, vpool, hpool, spool, tiny,
                      opool)


def unit_body(nc, u, xg, wg, yT,
              embT_s, WqT_s, WkT_s, WvT_s, WgT_s, c1n_s,
              bq_s, embb_s, cb_s, bvr, eps, one_b, ident, ones,
              ps, xpool, xepool, qkpool, vpool, hpool, spool, tiny, opool):
    ts = bass.ts

    # ---- load x (transposed) and per-token weights for this unit ----
    xT = xpool.tile([128, W, N], F32R, tag="xT")
    nc.sync.dma_start(xT, xg[u])
    wrow = xpool.tile([128, W, N], F32, tag="wrow")
    nc.gpsimd.dma_start(wrow, wg[u : u + 1].to_broadcast((128, W, N)))

    # ---- xe^T = emb(x)^T + emb_b   [128(d_in), 4(dchunk), W, N] ----
    xeT = xepool.tile([128, 4, W, N], F32R, tag="xeT")
    for cp in range(4):
        pxe = ps.tile([128, W, N], F32, tag="bank", name=f"pxe_{u}_{cp}")
        nc.tensor.matmul(pxe, embT_s[:, cp], xT, start=True, stop=True)
        nc.vector.tensor_scalar_add(xeT[:, cp], pxe, embb_s[:, cp : cp + 1])

    # ---- q^T (scaled, biased) and k^T, cast to bf16 ----
    qT = qkpool.tile([128, 4, W, N], BF16, tag="qT")
    kT = qkpool.tile([128, 4, W, N], BF16, tag="kT")
    for cp in range(4):
        pq = ps.tile([128, W, N], F32, tag="bank", name=f"pq_{u}_{cp}")
        for c in range(4):
            nc.tensor.matmul(pq, WqT_s[:, c, ts(cp, 128)], xeT[:, c],
                             start=(c == 0), stop=(c == 3))
        nc.vector.tensor_scalar_add(qT[:, cp], pq, bq_s[:, cp : cp + 1])
        pk = ps.tile([128, W, N], F32, tag="bank", name=f"pk_{u}_{cp}")
        for c in range(4):
            nc.tensor.matmul(pk, WkT_s[:, c, ts(cp, 128)], xeT[:, c],
                             start=(c == 0), stop=(c == 3))
        nc.scalar.copy(kT[:, cp], pk)

    # ---- v = xe @ wv_W.T + bv  [128(n), W, 512(d')] bf16 ----
    v = vpool.tile([128, W, 512], BF16, tag="v")
    for w_i in range(W):
        pv = ps.tile([128, 512], F32, tag="bank", name=f"pv_{u}_{w_i}")
        for c in range(4):
            nc.tensor.matmul(pv, xeT[:, c, w_i], WvT_s[:, c],
                             start=(c == 0), stop=(c == 3))
        nc.vector.tensor_add(v[:, w_i], pv, bvr)

    # ---- attention: scores + exp per example, combine unit-batched ----
    hT = hpool.tile([128, 4, W, N], F32R, tag="hT")
    e_all = spool.tile([128, W, H, N], F32, tag="e_all")
    for w_i in range(W):
        pss = ps.tile([128, H, N], F32, tag="bank", name=f"pss_{u}_{w_i}")
        for h in range(H):
            nc.tensor.matmul(pss[:, h], qT[:, h, w_i], kT[:, h, w_i],
                             start=True, stop=True)
        # no max-subtraction; scores are O(1) here
        nc.scalar.activation(e_all[:, w_i], pss, AF.Exp)
    # per-head row sums + normalizers, batched over the whole unit
    s_all = tiny.tile([128, W, H], F32, tag="s_all")
    nc.vector.reduce_sum(s_all, e_all, axis=AX.X)
    r_all = tiny.tile([128, W, H], F32, tag="r_all")
    nc.vector.reciprocal(r_all, s_all)
    # e <- e * r  (softmax per head); then sum heads; then token weights
    nc.vector.tensor_mul(e_all, e_all,
                         r_all[:, :, :, None].to_broadcast((128, W, H, N)))
    t1 = spool.tile([128, W, 2, N], F32, tag="t1")
    nc.vector.tensor_add(t1, e_all[:, :, 0:2], e_all[:, :, 2:4])
    Sw = spool.tile([128, W, N], F32, tag="Sw")
    nc.vector.tensor_add(Sw, t1[:, :, 0], t1[:, :, 1])
    nc.gpsimd.tensor_mul(Sw, Sw, wrow)
    dd = tiny.tile([128, W], F32, tag="dd")
    nc.vector.reduce_sum(dd, Sw, axis=AX.X)
    rd = tiny.tile([128, W], F32, tag="rd")
    nc.vector.reciprocal(rd, dd)
    Ab = spool.tile([128, W, N], BF16, tag="Ab")
    nc.vector.tensor_mul(Ab, Sw,
                         rd[:, :, None].to_broadcast((128, W, N)))
    for w_i in range(W):
        # attw^T via PE transpose (bf16)
        pT = ps.tile([128, N], BF16, tag="bank", name=f"pT_{u}_{w_i}")
        nc.tensor.transpose(pT, Ab[:, w_i], ident)
        awT = spool.tile([128, N], BF16, tag="awT")
        nc.vector.tensor_copy(awT, pT)
        # att_out^T = v^T-chunks @ attw^T   [128(d), 4(c), N]
        pa = ps.tile([128, 4, N], F32, tag="bank", name=f"pa_{u}_{w_i}")
        for c in range(4):
            nc.tensor.matmul(pa[:, c], v[:, w_i, ts(c, 128)], awT,
                             start=True, stop=True)
        # h^T = xe^T + softplus(att^T);  softplus(x) = ln(exp(x) + 1)
        ea = spool.tile([128, 4, N], F32, tag="ea")
        nc.scalar.activation(ea, pa, AF.Exp)
        sp = spool.tile([128, 4, N], F32, tag="sp")
        nc.scalar.activation(sp, ea, AF.Ln, bias=one_b)
        nc.gpsimd.tensor_add(hT[:, :, w_i], sp, xeT[:, :, w_i])

    # ---- LayerNorm stats over d (partition dim) via ones-matmul ----
    psum_s = ps.tile([128, W, N], F32, tag="bank", name=f"psum_s_{u}")
    psum_q = ps.tile([128, W, N], F32, tag="bank", name=f"psum_q_{u}")
    for c in range(4):
        nc.tensor.matmul(psum_s, ones, hT[:, c], start=(c == 0), stop=(c == 3))
        sqc = spool.tile([128, W, N], F32R, tag="sqc")
        nc.gpsimd.tensor_mul(sqc, hT[:, c], hT[:, c])
        nc.tensor.matmul(psum_q, ones, sqc, start=(c == 0), stop=(c == 3))
    # mu2 = (s/512)^2 ; var = q/512 - mu2
    # rstd = 1/sqrt(var+eps) = exp(-0.5*ln(var+eps))  (stays in the
    # exp/ln/square activation table set - no table reloads)
    mu2 = spool.tile([128, W, N], F32, tag="mu2")
    nc.scalar.activation(mu2, psum_s, AF.Square, scale=1.0 / D)
    var = spool.tile([128, W, N], F32, tag="var")
    nc.vector.scalar_tensor_tensor(out=var, in0=psum_q, scalar=1.0 / D,
                                   in1=mu2, op0=OP.mult, op1=OP.subtract)
    lv = spool.tile([128, W, N], F32, tag="lv")
    nc.scalar.activation(lv, var, AF.Ln, bias=eps)
    rstd = spool.tile([128, W, N], F32, tag="rstd")
    nc.scalar.activation(rstd, lv, AF.Exp, scale=-0.5)
    # m2 = mu   (row 0 feeds the K=1 correction matmul; final *rstd
    # multiplies the whole PSUM including this term)
    m2 = spool.tile([128, W, N], F32R, tag="m2")
    nc.vector.tensor_scalar_mul(m2, psum_s, 1.0 / D)

    # ---- out^T = Wg-chunks @ h^T - c1 x m2 ;  then *rstd + cb ----
    po = ps.tile([128, W, N], F32, tag="bank", name=f"po_{u}")
    for c in range(4):
        nc.tensor.matmul(po, WgT_s[:, c], hT[:, c],
                         start=(c == 0), stop=False)
    nc.tensor.matmul(po, c1n_s, m2[0:1], start=False, stop=True)
    outT = opool.tile([128, W, N], F32, tag="outT")
    nc.vector.tensor_mul(outT, po, rstd)
    nc.scalar.activation(outT, outT, AF.Identity, bias=cb_s)
    nc.sync.dma_start(yT[u], outT)


# ------------------------- host side -------------------------

def host_prep(x, weights, emb_W, emb_b, wq_W, wq_b, wk_W, wk_b, wv_W, wv_b,
              in_proj_W, in_proj_b, ln_g, ln_b, out_W, out_b):
    """Fuse/reshape parameters and build per-core input maps."""
    f = np.float32
    sc = 1.0 / np.sqrt(np.float32(E))

    Wq = in_proj_W[:D]
    Wk = in_proj_W[D : 2 * D]
    bqi = in_proj_b[:D]
    Wqc = (Wq @ wq_W) * sc
    bq_eff = (Wq @ wq_b + bqi) * sc
    Wkc = Wk @ wk_W

    Wg = out_W.T * ln_g[:, None]          # [D, E]
    c1 = Wg.sum(axis=0)                   # [E]
    cbv = out_b + out_W @ ln_b            # [E]

    def chunk_T(M):
        # M is [D_in, D_out] applied as x @ M; -> [128, 4, D_out]
        Dout = M.shape[1]
        return np.ascontiguousarray(
            M.reshape(4, 128, Dout).transpose(1, 0, 2)).astype(f)

    def pcol(vec):
        return np.ascontiguousarray(vec.reshape(4, 128).T).astype(f)

    params = {
        "embT": np.ascontiguousarray(emb_W.T.reshape(128, 4, 128)).astype(f),
        "WqT": chunk_T(Wqc.T),
        "WkT": chunk_T(Wkc.T),
        "WvT": chunk_T(wv_W.T),
        "WgT": chunk_T(Wg),
        "c1n": np.ascontiguousarray((-c1).reshape(1, 128)).astype(f),
        "bq": pcol(bq_eff),
        "embb": pcol(emb_b),
        "cb": np.ascontiguousarray(cbv.reshape(128, 1)).astype(f),
        "bv": np.ascontiguousarray(wv_b.reshape(1, 512)).astype(f),
        "onesm": np.ones((128, 128), dtype=f),
    }
    import ml_dtypes
    params["idm"] = np.eye(128).astype(ml_dtypes.bfloat16)

    in_maps = []
    for c in range(NCORES):
        xs = x[c * BC : (c + 1) * BC].astype(f)
        ws = weights[c * BC : (c + 1) * BC, :, 0].astype(f)
        xgc = np.ascontiguousarray(
            xs.reshape(NU, W, N, E).transpose(0, 3, 1, 2))
        wgc = np.ascontiguousarray(ws.reshape(NU, W, N))
        m = dict(params)
        m["xg"] = xgc
        m["wg"] = wgc
        in_maps.append(m)
    return in_maps


_NC_CACHE = {}


def kernel(**inputs):
    key = "full"
    if key not in _NC_CACHE:
        _NC_CACHE[key] = build_nc(NU)
    nc = _NC_CACHE[key]
    in_maps = host_prep(**inputs)
    res = run_bass_kernel_spmd(nc, in_maps, core_ids=list(range(NCORES)))
    outs = []
    for c in range(NCORES):
        yt = res.results[c]["yT"]                  # [NU, 128(E), W, N]
        y = yt.transpose(0, 2, 3, 1).reshape(BC, N, E)
        outs.append(y)
    return np.ascontiguousarray(np.concatenate(outs, axis=0)).astype(np.float32)


# revision 61
# speedup vs baseline: 2.2482x; 1.4607x over previous
"""Trainium2 Bass kernel for nn_PeriodicSetTransformerEncoder.

Math (per example, N=128 tokens, E=128, D=512, H=4 heads, head_dim=128):
  xe   = x @ emb_W.T + emb_b                       [N, D]
  q    = xe @ Wqc.T + bq_eff    (Wqc = Wq@wq_W, scaled by 1/sqrt(hd))
  k    = xe @ Wkc.T             (k bias drops out of softmax)
  v    = xe @ wv_W.T + bv_eff
  s_h  = q_h @ k_h.T            per head            [N, N]
  e_h  = exp(s_h);  attw ~ (sum_h e_h/rowsum(e_h)) * w[j], renormalized
  att  = attw @ v
  h    = xe + softplus(att);  out = LN(h)*g+b @ out_W.T + out_b

LayerNorm affine + normalization is folded into the output projection:
  out[e,n] = rstd[n]*(Wg.T h)[e,n] - mu[n]*rstd[n]*c1[e] + cb[e]
with Wg = out_W.T * g (host-fused), c1[e] = sum_d Wg[d,e],
cb[e] = out_b[e] + sum_d out_W.T[d,e]*ln_b[d].  The -mu*rstd*c1 term is
one extra K=1 matmul accumulated into the same PSUM bank.

The key_padding_mask in the reference is all-False for these inputs
(xe rows are never exactly all-zero with gaussian x), so it is skipped
(verified in test.py).

Sharding: pure data parallel, batch 512 -> 64 examples on each of 8 cores.
Device layout: features on partitions, tokens on the free dim, so all big
matmuls have moving free dim 512 (4 examples x 128 tokens) and run
float32r at 1 cycle/row.
"""

import numpy as np

import concourse.bass as bass
import concourse.tile as tile
from concourse import bacc, mybir
from concourse.bass_utils import run_bass_kernel_spmd

F32 = mybir.dt.float32
F32R = mybir.dt.float32r
BF16 = mybir.dt.bfloat16
AX = mybir.AxisListType
OP = mybir.AluOpType
AF = mybir.ActivationFunctionType

B = 512
N = 128
E = 128
D = 512
H = 4
NCORES = 8
BC = B // NCORES          # examples per core
W = 4                     # examples per work unit (free-dim batching)
NU = BC // W              # work units per core


def build_nc(nu=NU):
    nc = bacc.Bacc("TRN2", target_bir_lowering=False, debug=False)

    xg = nc.dram_tensor("xg", [nu, 128, W, N], F32R, kind="ExternalInput").ap()
    wg = nc.dram_tensor("wg", [nu, W, N], F32, kind="ExternalInput").ap()
    embT = nc.dram_tensor("embT", [128, 4, 128], F32R, kind="ExternalInput").ap()
    WqT = nc.dram_tensor("WqT", [128, 4, 512], F32R, kind="ExternalInput").ap()
    WkT = nc.dram_tensor("WkT", [128, 4, 512], F32R, kind="ExternalInput").ap()
    WvT = nc.dram_tensor("WvT", [128, 4, 512], F32R, kind="ExternalInput").ap()
    WgT = nc.dram_tensor("WgT", [128, 4, 128], F32R, kind="ExternalInput").ap()
    c1n = nc.dram_tensor("c1n", [1, 128], F32R, kind="ExternalInput").ap()
    bq = nc.dram_tensor("bq", [128, 4], F32, kind="ExternalInput").ap()
    embb = nc.dram_tensor("embb", [128, 4], F32, kind="ExternalInput").ap()
    cb = nc.dram_tensor("cb", [128, 1], F32, kind="ExternalInput").ap()
    bv = nc.dram_tensor("bv", [1, 512], F32, kind="ExternalInput").ap()
    idm = nc.dram_tensor("idm", [128, 128], BF16, kind="ExternalInput").ap()
    onesm = nc.dram_tensor("onesm", [128, 128], F32R, kind="ExternalInput").ap()
    yT = nc.dram_tensor("yT", [nu, 128, W, N], F32, kind="ExternalOutput").ap()

    with tile.TileContext(nc) as tc:
        kernel_body(tc, nu, xg, wg, embT, WqT, WkT, WvT, WgT, c1n,
                    bq, embb, cb, bv, idm, onesm, yT)
    # All transcendentals here (exp/ln/square) live in the
    # "natural_log_exp_and_others" activation table set.  Restrict the
    # table map during compile so the act-table-load pass emits a single
    # load instead of thrashing between per-function sets.
    from concourse import hw_specs
    orig = hw_specs.get_activation_tables

    def patched(arch):
        t = orig(arch)
        strip = {AF.Exp, AF.Ln, AF.Square}
        for name, fs in t.items():
            if name != "natural_log_exp_and_others":
                t[name] = fs - strip
        return t

    hw_specs.get_activation_tables = patched
    bacc_mod = __import__("concourse.bacc", fromlist=["get_activation_tables"])
    had = getattr(bacc_mod, "get_activation_tables", None)
    if had is not None:
        bacc_mod.get_activation_tables = patched
    try:
        nc.compile()
    finally:
        hw_specs.get_activation_tables = orig
        if had is not None:
            bacc_mod.get_activation_tables = had
    return nc


def kernel_body(tc, nu, xg, wg, embT, WqT, WkT, WvT, WgT, c1n,
                bq, embb, cb, bv, idm, onesm, yT):
    nc = tc.nc
    from contextlib import ExitStack
    ctx = ExitStack()
    with ctx:
        const = ctx.enter_context(tc.tile_pool(name="const", bufs=1))
        ps = ctx.enter_context(tc.tile_pool(name="ps", bufs=3, space="PSUM"))
        psa = ctx.enter_context(tc.tile_pool(name="psa", bufs=2, space="PSUM"))
        pso = ctx.enter_context(tc.tile_pool(name="pso", bufs=3, space="PSUM"))
        xpool = ctx.enter_context(tc.tile_pool(name="xpool", bufs=3))
        xepool = ctx.enter_context(tc.tile_pool(name="xepool", bufs=3))
        qkpool = ctx.enter_context(tc.tile_pool(name="qkpool", bufs=3))
        vpool = ctx.enter_context(tc.tile_pool(name="vpool", bufs=2))
        hpool = ctx.enter_context(tc.tile_pool(name="hpool", bufs=2))
        spool = ctx.enter_context(tc.tile_pool(name="spool", bufs=2))
        sxpool = ctx.enter_context(tc.tile_pool(name="sxpool", bufs=4))
        tiny = ctx.enter_context(tc.tile_pool(name="tiny", bufs=3))
        opool = ctx.enter_context(tc.tile_pool(name="opool", bufs=2))

        # ---- constants ----
        ident = const.tile([128, 128], BF16)
        nc.sync.dma_start(ident, idm)
        ones = const.tile([128, 128], F32R)
        nc.sync.dma_start(ones, onesm)
        embT_s = const.tile([128, 4, 128], F32R)
        nc.sync.dma_start(embT_s, embT)
        WqT_s = const.tile([128, 4, 512], F32R)
        nc.sync.dma_start(WqT_s, WqT)
        WkT_s = const.tile([128, 4, 512], F32R)
        nc.sync.dma_start(WkT_s, WkT)
        WvT_s = const.tile([128, 4, 512], F32R)
        nc.sync.dma_start(WvT_s, WvT)
        WgT_s = const.tile([128, 4, 128], F32R)
        nc.sync.dma_start(WgT_s, WgT)
        c1n_s = const.tile([1, 128], F32R)
        nc.sync.dma_start(c1n_s, c1n)
        bq_s = const.tile([128, 4], F32)
        nc.sync.dma_start(bq_s, bq)
        embb_s = const.tile([128, 4], F32)
        nc.sync.dma_start(embb_s, embb)
        cb_s = const.tile([128, 1], F32)
        nc.sync.dma_start(cb_s, cb)
        bvr = const.tile([128, 512], F32)
        nc.gpsimd.dma_start(bvr, bv.to_broadcast((128, 512)))
        eps = const.tile([128, 1], F32)
        nc.vector.memset(eps, 1e-5)
        one_b = const.tile([128, 1], F32)
        nc.vector.memset(one_b, 1.0)

        for u in range(nu):
            unit_body(nc, u, xg, wg, yT,
                      embT_s, WqT_s, WkT_s, WvT_s, WgT_s, c1n_s,
                      bq_s, embb_s, cb_s, bvr, eps, one_b, ident, ones,
                      ps, psa, pso, xpool, xepool, qkpool, vpool, hpool,
                      spool, sxpool, tiny, opool)


def unit_body(nc, u, xg, wg, yT,
              embT_s, WqT_s, WkT_s, WvT_s, WgT_s, c1n_s,
              bq_s, embb_s, cb_s, bvr, eps, one_b, ident, ones,
              ps, psa, pso, xpool, xepool, qkpool, vpool, hpool,
              spool, sxpool, tiny, opool):
    ts = bass.ts

    # ---- load x (transposed) and per-token weights for this unit ----
    xT = xpool.tile([128, W, N], F32R, tag="xT")
    nc.sync.dma_start(xT, xg[u])
    wrow = xpool.tile([128, W, N], F32, tag="wrow")
    nc.gpsimd.dma_start(wrow, wg[u : u + 1].to_broadcast((128, W, N)))

    # ---- xe^T = emb(x)^T + emb_b   [128(d_in), 4(dchunk), W, N] ----
    xeT = xepool.tile([128, 4, W, N], F32R, tag="xeT")
    for cp in range(4):
        pxe = ps.tile([128, W, N], F32, tag="bank", name=f"pxe_{u}_{cp}")
        nc.tensor.matmul(pxe, embT_s[:, cp], xT, start=True, stop=True)
        nc.vector.tensor_scalar_add(xeT[:, cp], pxe, embb_s[:, cp : cp + 1])

    # ---- q^T (scaled, biased) and k^T, cast to bf16 ----
    qT = qkpool.tile([128, 4, W, N], BF16, tag="qT")
    kT = qkpool.tile([128, 4, W, N], BF16, tag="kT")
    for cp in range(4):
        pq = ps.tile([128, W, N], F32, tag="bank", name=f"pq_{u}_{cp}")
        for c in range(4):
            nc.tensor.matmul(pq, WqT_s[:, c, ts(cp, 128)], xeT[:, c],
                             start=(c == 0), stop=(c == 3))
        nc.vector.tensor_scalar_add(qT[:, cp], pq, bq_s[:, cp : cp + 1])
        pk = ps.tile([128, W, N], F32, tag="bank", name=f"pk_{u}_{cp}")
        for c in range(4):
            nc.tensor.matmul(pk, WkT_s[:, c, ts(cp, 128)], xeT[:, c],
                             start=(c == 0), stop=(c == 3))
        nc.scalar.copy(kT[:, cp], pk)

    # ---- v = xe @ wv_W.T + bv  [128(n), W, 512(d')] bf16 ----
    v = vpool.tile([128, W, 512], BF16, tag="v")
    for w_i in range(W):
        pv = ps.tile([128, 512], F32, tag="bank", name=f"pv_{u}_{w_i}")
        for c in range(4):
            nc.tensor.matmul(pv, xeT[:, c, w_i], WvT_s[:, c],
                             start=(c == 0), stop=(c == 3))
        nc.vector.tensor_add(v[:, w_i], pv, bvr)

    # ---- attention: scores + exp per example, combine unit-batched ----
    hT = hpool.tile([128, 4, W, N], F32R, tag="hT")
    e_all = spool.tile([128, W, H, N], F32, tag="e_all")
    for w_i in range(W):
        pss = psa.tile([128, H, N], F32, tag="bank", name=f"pss_{u}_{w_i}")
        for h in range(H):
            nc.tensor.matmul(pss[:, h], qT[:, h, w_i], kT[:, h, w_i],
                             start=True, stop=True)
        # no max-subtraction; scores are O(1) here
        nc.scalar.activation(e_all[:, w_i], pss, AF.Exp)
    # per-head row sums + normalizers, batched over the whole unit
    s_all = tiny.tile([128, W, H], F32, tag="s_all")
    nc.vector.reduce_sum(s_all, e_all, axis=AX.X)
    r_all = tiny.tile([128, W, H], F32, tag="r_all")
    nc.vector.reciprocal(r_all, s_all)
    # e <- e * r  (softmax per head); then sum heads; then token weights
    nc.vector.tensor_mul(e_all, e_all,
                         r_all[:, :, :, None].to_broadcast((128, W, H, N)))
    nc.vector.tensor_add(e_all[:, :, 0:2], e_all[:, :, 0:2],
                         e_all[:, :, 2:4])
    Sw = spool.tile([128, W, N], F32, tag="Sw")
    nc.vector.tensor_add(Sw, e_all[:, :, 0], e_all[:, :, 1])
    nc.gpsimd.tensor_mul(Sw, Sw, wrow)
    dd = tiny.tile([128, W], F32, tag="dd")
    nc.vector.reduce_sum(dd, Sw, axis=AX.X)
    rd = tiny.tile([128, W], F32, tag="rd")
    nc.vector.reciprocal(rd, dd)
    Ab = spool.tile([128, W, N], BF16, tag="Ab")
    nc.vector.tensor_mul(Ab, Sw,
                         rd[:, :, None].to_broadcast((128, W, N)))
    for w_i in range(W):
        # attw^T via PE transpose (bf16)
        pT = psa.tile([128, N], BF16, tag="bank", name=f"pT_{u}_{w_i}")
        nc.tensor.transpose(pT, Ab[:, w_i], ident)
        awT = sxpool.tile([128, N], BF16, tag="awT")
        nc.vector.tensor_copy(awT, pT)
        # att_out^T = v^T-chunks @ attw^T   [128(d), 4(c), N]
        pa = psa.tile([128, 4, N], F32, tag="bank", name=f"pa_{u}_{w_i}")
        for c in range(4):
            nc.tensor.matmul(pa[:, c], v[:, w_i, ts(c, 128)], awT,
                             start=True, stop=True)
        # h^T = xe^T + softplus(att^T);  softplus(x) = ln(exp(x) + 1)
        ea = sxpool.tile([128, 4, N], F32, tag="ea")
        nc.scalar.activation(ea, pa, AF.Exp)
        nc.scalar.activation(ea, ea, AF.Ln, bias=one_b)
        nc.gpsimd.tensor_add(hT[:, :, w_i], ea, xeT[:, :, w_i])

    # ---- LayerNorm stats over d (partition dim) via ones-matmul ----
    psum_s = pso.tile([128, W, N], F32, tag="bank", name=f"psum_s_{u}")
    psum_q = pso.tile([128, W, N], F32, tag="bank", name=f"psum_q_{u}")
    for c in range(4):
        nc.tensor.matmul(psum_s, ones, hT[:, c], start=(c == 0), stop=(c == 3))
        sqc = sxpool.tile([128, W, N], F32R, tag="sqc")
        nc.gpsimd.tensor_mul(sqc, hT[:, c], hT[:, c])
        nc.tensor.matmul(psum_q, ones, sqc, start=(c == 0), stop=(c == 3))
    # mu2 = (s/512)^2 ; var = q/512 - mu2
    # rstd = 1/sqrt(var+eps) = exp(-0.5*ln(var+eps))  (stays in the
    # exp/ln/square activation table set - no table reloads)
    mu2 = spool.tile([128, W, N], F32, tag="mu2")
    nc.scalar.activation(mu2, psum_s, AF.Square, scale=1.0 / D)
    var = spool.tile([128, W, N], F32, tag="var")
    nc.vector.scalar_tensor_tensor(out=var, in0=psum_q, scalar=1.0 / D,
                                   in1=mu2, op0=OP.mult, op1=OP.subtract)
    lv = spool.tile([128, W, N], F32, tag="lv")
    nc.scalar.activation(lv, var, AF.Ln, bias=eps)
    rstd = spool.tile([128, W, N], F32, tag="rstd")
    nc.scalar.activation(rstd, lv, AF.Exp, scale=-0.5)
    # m2 = mu   (row 0 feeds the K=1 correction matmul; final *rstd
    # multiplies the whole PSUM including this term)
    m2 = spool.tile([128, W, N], F32R, tag="m2")
    nc.vector.tensor_scalar_mul(m2, psum_s, 1.0 / D)

    # ---- out^T = Wg-chunks @ h^T - c1 x m2 ;  then *rstd + cb ----
    po = pso.tile([128, W, N], F32, tag="bank", name=f"po_{u}")
    for c in range(4):
        nc.tensor.matmul(po, WgT_s[:, c], hT[:, c],
                         start=(c == 0), stop=False)
    nc.tensor.matmul(po, c1n_s, m2[0:1], start=False, stop=True)
    outT = opool.tile([128, W, N], F32, tag="outT")
    nc.vector.tensor_mul(outT, po, rstd)
    nc.scalar.activation(outT, outT, AF.Identity, bias=cb_s)
    nc.sync.dma_start(yT[u], outT)


# ------------------------- host side -------------------------

def host_prep(x, weights, emb_W, emb_b, wq_W, wq_b, wk_W, wk_b, wv_W, wv_b,
              in_proj_W, in_proj_b, ln_g, ln_b, out_W, out_b):
    """Fuse/reshape parameters and build per-core input maps."""
    f = np.float32
    sc = 1.0 / np.sqrt(np.float32(E))

    Wq = in_proj_W[:D]
    Wk = in_proj_W[D : 2 * D]
    bqi = in_proj_b[:D]
    Wqc = (Wq @ wq_W) * sc
    bq_eff = (Wq @ wq_b + bqi) * sc
    Wkc = Wk @ wk_W

    Wg = out_W.T * ln_g[:, None]          # [D, E]
    c1 = Wg.sum(axis=0)                   # [E]
    cbv = out_b + out_W @ ln_b            # [E]

    def chunk_T(M):
        # M is [D_in, D_out] applied as x @ M; -> [128, 4, D_out]
        Dout = M.shape[1]
        return np.ascontiguousarray(
            M.reshape(4, 128, Dout).transpose(1, 0, 2)).astype(f)

    def pcol(vec):
        return np.ascontiguousarray(vec.reshape(4, 128).T).astype(f)

    params = {
        "embT": np.ascontiguousarray(emb_W.T.reshape(128, 4, 128)).astype(f),
        "WqT": chunk_T(Wqc.T),
        "WkT": chunk_T(Wkc.T),
        "WvT": chunk_T(wv_W.T),
        "WgT": chunk_T(Wg),
        "c1n": np.ascontiguousarray((-c1).reshape(1, 128)).astype(f),
        "bq": pcol(bq_eff),
        "embb": pcol(emb_b),
        "cb": np.ascontiguousarray(cbv.reshape(128, 1)).astype(f),
        "bv": np.ascontiguousarray(wv_b.reshape(1, 512)).astype(f),
        "onesm": np.ones((128, 128), dtype=f),
    }
    import ml_dtypes
    params["idm"] = np.eye(128).astype(ml_dtypes.bfloat16)

    in_maps = []
    for c in range(NCORES):
        xs = x[c * BC : (c + 1) * BC].astype(f)
        ws = weights[c * BC : (c + 1) * BC, :, 0].astype(f)
        xgc = np.ascontiguousarray(
            xs.reshape(NU, W, N, E).transpose(0, 3, 1, 2))
        wgc = np.ascontiguousarray(ws.reshape(NU, W, N))
        m = dict(params)
        m["xg"] = xgc
        m["wg"] = wgc
        in_maps.append(m)
    return in_maps


_NC_CACHE = {}


def kernel(**inputs):
    key = "full"
    if key not in _NC_CACHE:
        _NC_CACHE[key] = build_nc(NU)
    nc = _NC_CACHE[key]
    in_maps = host_prep(**inputs)
    res = run_bass_kernel_spmd(nc, in_maps, core_ids=list(range(NCORES)))
    outs = []
    for c in range(NCORES):
        yt = res.results[c]["yT"]                  # [NU, 128(E), W, N]
        y = yt.transpose(0, 2, 3, 1).reshape(BC, N, E)
        outs.append(y)
    return np.ascontiguousarray(np.concatenate(outs, axis=0)).astype(np.float32)
